# revision 23
# baseline (speedup 1.0000x reference)
"""De-stationary attention on 8 Trainium2 NeuronCores.

Problem: y = softmax((x Wq^T + bq)(x Wk^T + bk)^T * scale / (tau*x_std)) (x Wv^T + bv) Wo^T + bo
Shapes: x [4, 2048, 1024], 16 heads of 64 dims, tau=1, delta=0.

Sharding: core c handles batch b = c//2, head group g = c%2 (8 heads).
The de-stationary scale s = SCALE/x_std[b] is folded into Wq on the host.
Each core computes a partial y over its 8 heads; host sums the two
head-group partials per batch and adds bo + bv @ Wo.T (value bias passes
through softmax rows which sum to 1).

Device math per core (all matmuls float32r = TF32-like, 1 cyc/row):
  qT [512, 2048] = (s*Wq_g) x^T + s*bq  (head-pair tiles [128, 2048])
  kT [512, 2048] = Wk_g x^T + bk
  v  [2048, 8*65] : per head 64 value cols + a ones col (gives the
     softmax denominator as row 64 of the PV matmul output)
  per head pair, per tq half [1024], per tk tile [128]:
     S^T[tk, tq] = kT_h^T-slice . qT_h  (K=64, even head on PE rows 0-63,
                   odd head on rows 64-127 -> concurrent via row groups)
     P^T = exp(S^T)    (no max subtraction: |logits| <= ~25, fp32 safe)
     psum_O[65, tq] += v_aug[tk]^T . P^T   (row 64 accumulates l = sum P)
  O_norm = psum_O[0:64] * (1/l) broadcast (DVE recip + DRAM-bounce bcast)
  y_part[2048, 1024] = sum_pairs O_pair^T . wo_pair  (K=128)
"""

import os
import sys

for _p in ("/opt/trn_rl_repo", "/root/.axon_site/_ro/trn_rl_repo"):
    if os.path.isdir(_p) and _p not in sys.path:
        sys.path.insert(0, _p)

import numpy as np

import concourse.bass as bass
import concourse.mybir as mybir
import concourse.tile as tile
from concourse import bacc
from concourse.bass_utils import run_bass_kernel_spmd

F32 = mybir.dt.float32
F32R = mybir.dt.float32r
BF16 = mybir.dt.bfloat16
AF = mybir.ActivationFunctionType

B, T, D, H = 4, 2048, 1024, 16
HD = D // H          # 64
SCALE = HD ** -0.5
HG = H // 2          # 8 heads per core
EG = HG * HD         # 512 projection dims per core
N_CORES = 8

TQH = T // 2         # 1024: tq half processed per attention inner loop
NTK = T // 128       # 16 tk tiles
NPAIR = HG // 2      # 4 head pairs per core


def _build():
    nc = bacc.Bacc("TRN2", target_bir_lowering=False, debug=False)

    xt_d = nc.dram_tensor("xt", [D, T], F32, kind="ExternalInput")
    wq_d = nc.dram_tensor("wq", [D, EG], F32, kind="ExternalInput")
    wk_d = nc.dram_tensor("wk", [D, EG], F32, kind="ExternalInput")
    wv_d = nc.dram_tensor("wv", [D, EG], F32, kind="ExternalInput")
    wo_d = nc.dram_tensor("wo", [EG, D], F32, kind="ExternalInput")
    bq_d = nc.dram_tensor("bq", [EG], F32, kind="ExternalInput")
    # mask[:, 0] = rows 0:64 one / rows 64:128 zero; mask[:, 1] inverted.
    # bk needs no device handling: the q.bk logit term is constant along
    # the key axis and cancels in softmax.
    mk_d = nc.dram_tensor("mask", [128, 2], F32, kind="ExternalInput")
    y_d = nc.dram_tensor("y", [T, D], F32, kind="ExternalOutput")
    DBG = os.environ.get("KDBG", "0") == "1"
    if DBG:
        dbg_q = nc.dram_tensor("dbg_q", [128, T], F32, kind="ExternalOutput")
        dbg_k = [nc.dram_tensor(f"dbg_k{h}", [128, T], F32,
                                kind="ExternalOutput") for h in range(2)]
        dbg_o = nc.dram_tensor("dbg_o", [128, T], F32, kind="ExternalOutput")
    # scratch for the l / 1/l bounces (per head-pair iteration, per head)
    ls_d = nc.dram_tensor("l_scratch", [NPAIR, 2, 2, TQH], F32)
    rs_d = nc.dram_tensor("rinv_scratch", [NPAIR, 2, 2, TQH], F32)

    with tile.TileContext(nc) as tc:
        from contextlib import ExitStack
        with ExitStack() as octx:
            # ---- persistent pools (live through phases A-C) ----
            qk_pool = octx.enter_context(tc.tile_pool(name="qk", bufs=1))
            v_pool = octx.enter_context(tc.tile_pool(name="vp", bufs=1))

            qT = [qk_pool.tile([128, T], F32R, name=f"qT{j}", tag=f"qT{j}")
                  for j in range(NPAIR)]

            # per-head kT, zero-padded to K=128: head h occupies rows
            # (h%2)*64:(h%2+1)*64, other 64 rows are zero. Zero-padding keeps
            # the S^T matmuls at K=128 / no row-groups: K=64 row-group
            # matmuls do not register as PE activity for the HAM clock
            # governor, which then throttles the PE to 1.2 GHz.
            kT = [qk_pool.tile([128, T], F32R, name=f"kT{h}", tag=f"kT{h}")
                  for h in range(HG)]
            v_sb = [v_pool.tile([128, HG * (HD + 1)], F32R, name=f"v{t}",
                                tag=f"v{t}") for t in range(NTK)]

            # ---- warm-up: trip the PE HAM to full clock while DMAs land
            with tc.tile_pool(name="wu", bufs=1) as wup, \
                 tc.tile_pool(name="wu_ps", bufs=1, space="PSUM") as wups:
                wu_t = wup.tile([128, 512], F32R, name="wu")
                nc.vector.memset(wu_t[:].bitcast(F32), 0.0)
                wu_ps = wups.tile([128, 512], F32, name="wups")
                for i in range(80):
                    nc.tensor.matmul(wu_ps[:], wu_t[:, 0:128], wu_t[:],
                                     start=True, stop=True,
                                     skip_group_check=True)

            # ---- phase A: projections ----
            with tc.tile_pool(name="pa", bufs=1) as pa, \
                 tc.tile_pool(name="pa_x", bufs=1) as pax, \
                 tc.tile_pool(name="pa_ps", bufs=4, space="PSUM") as paps:
                wq_t = [pa.tile([128, EG], F32R, name=f"wq{k}", tag=f"wq{k}")
                        for k in range(8)]
                wk_t = [pa.tile([128, EG], F32R, name=f"wk{k}", tag=f"wk{k}")
                        for k in range(8)]
                wv_t = [pa.tile([128, EG], F32R, name=f"wv{k}", tag=f"wv{k}")
                        for k in range(8)]
                bq_t = [pa.tile([128, 1], F32, name=f"bq{e}", tag=f"bq{e}")
                        for e in range(4)]
                mk_t = pa.tile([128, 2], F32, name="mk")
                nc.sync.dma_start(mk_t[:], mk_d.ap())
                for k in range(8):
                    r = slice(k * 128, (k + 1) * 128)
                    nc.sync.dma_start(wq_t[k][:], wq_d.ap()[r, :].bitcast(F32R))
                for e in range(4):
                    nc.sync.dma_start(bq_t[e][:],
                                      bq_d.ap()[e * 128:(e + 1) * 128])
                for k in range(8):
                    r = slice(k * 128, (k + 1) * 128)
                    nc.sync.dma_start(wk_t[k][:], wk_d.ap()[r, :].bitcast(F32R))
                for k in range(8):
                    r = slice(k * 128, (k + 1) * 128)
                    nc.sync.dma_start(wv_t[k][:], wv_d.ap()[r, :].bitcast(F32R))

                for tq in range(4):          # t quarters of 512
                    tsl = slice(tq * 512, (tq + 1) * 512)
                    x_t = [pax.tile([128, 512], F32R, name=f"x{k}", tag=f"x{k}",
                                    bufs=(2 if k < 7 else 1))
                           for k in range(8)]
                    for k in range(8):
                        nc.sync.dma_start(
                            x_t[k][:],
                            xt_d.ap()[k * 128:(k + 1) * 128, tsl].bitcast(F32R))
                    # q projection: out [e-tile 128, t 512], bias bq
                    for e in range(4):
                        ps = paps.tile([128, 512], F32, name="pj", tag="pj")
                        esl = slice(e * 128, (e + 1) * 128)
                        for k in range(8):
                            nc.tensor.matmul(ps[:], wq_t[k][:, esl], x_t[k][:],
                                             start=(k == 0), stop=(k == 7))
                        nc.vector.tensor_scalar_add(qT[e][:, tsl], ps[:],
                                                    bq_t[e][:])
                    # k projection -> per-head zero-padded tiles via masks
                    for e in range(4):
                        ps = paps.tile([128, 512], F32, name="pj", tag="pj")
                        esl = slice(e * 128, (e + 1) * 128)
                        for k in range(8):
                            nc.tensor.matmul(ps[:], wk_t[k][:, esl], x_t[k][:],
                                             start=(k == 0), stop=(k == 7))
                        for p in range(2):
                            nc.vector.tensor_scalar(
                                kT[2 * e + p][:, tsl], ps[:],
                                mk_t[:, p:p + 1], None,
                                mybir.AluOpType.mult)
                    # v projection: out [tk-tile 128, e 512] -> 65-strided
                    for tt in range(4):
                        ps = paps.tile([128, 512], F32, name="pj", tag="pj")
                        ti = tq * 4 + tt
                        for k in range(8):
                            nc.tensor.matmul(
                                ps[:], x_t[k][:, tt * 128:(tt + 1) * 128],
                                wv_t[k][:], start=(k == 0), stop=(k == 7))
                        vre = v_sb[ti][:].rearrange("p (h c) -> p h c", c=HD + 1)
                        nc.vector.tensor_copy(
                            vre[:, :, 0:HD],
                            ps[:].rearrange("p (h c) -> p h c", c=HD))
                        nc.scalar.activation(vre[:, :, HD:HD + 1],
                                             ps[:].rearrange(
                                                 "p (h c) -> p h c", c=HD)[:, :, 0:1],
                                             AF.Identity, bias=1.0, scale=0.0)

            # ---- phases B+C pools ----
            with tc.tile_pool(name="pb", bufs=1) as pb:
                wo_t = [pb.tile([128, D], F32R, name=f"wo{j}", tag=f"wo{j}")
                        for j in range(NPAIR)]
                for j in range(NPAIR):
                    nc.sync.dma_start(
                        wo_t[j][:],
                        wo_d.ap()[j * 128:(j + 1) * 128, :].bitcast(F32R))
                o_sb = [pb.tile([128, T], F32R, name=f"o{j}", tag=f"o{j}")
                        for j in range(NPAIR)]

                # ---- phase B: attention ----
                pbps_ctx = tc.tile_pool(name="pb_ps", bufs=1, space="PSUM")
                pbps = pbps_ctx.__enter__()
                for j in range(NPAIR):
                    for th in range(2):
                        qsl = slice(th * TQH, (th + 1) * TQH)
                        ps_o = [pbps.tile([65, TQH], F32, name=f"po{p}",
                                          tag=f"po{p}", bufs=1)
                                for p in range(2)]
                        for tk in range(NTK):
                            ksl = slice(tk * 128, (tk + 1) * 128)
                            # S matmuls for both heads adjacent: even head on
                            # PE rows 0-63, odd on 64-127 -> run concurrently
                            ps_s = [pbps.tile([128, TQH], F32, name=f"ps{p}",
                                              tag=f"ps{p}", bufs=1)
                                    for p in range(2)]
                            for p in range(2):
                                for nk in range(2):
                                    nsl = slice(nk * 512, (nk + 1) * 512)
                                    nc.tensor.matmul(
                                        ps_s[p][:, nsl],
                                        kT[2 * j + p][:, ksl],
                                        qT[j][:, qsl][:, nsl],
                                        start=True, stop=True)
                            pt = [None, None]
                            for p in range(2):
                                pt[p] = pb.tile([128, TQH], F32R,
                                                name=f"pt{p}", tag="pt",
                                                bufs=3)
                                nc.scalar.activation(pt[p][:], ps_s[p][:], AF.Exp)
                            for p in range(2):
                                h = 2 * j + p
                                vcol = slice(h * (HD + 1), (h + 1) * (HD + 1))
                                for nk in range(2):
                                    nsl = slice(nk * 512, (nk + 1) * 512)
                                    nc.tensor.matmul(
                                        ps_o[p][:, nsl], v_sb[tk][:, vcol],
                                        pt[p][:, nsl],
                                        start=(tk == 0), stop=(tk == NTK - 1))
                        # normalize: O = psum_O[0:64] * (1/l), l = row 64.
                        # l goes SBUF->DRAM->[64,16] sbuf, recip there (fast:
                        # 16 elems/partition), back out, then bcast [64, TQH].
                        rbc = pb.tile([128, TQH], F32, name="rbc", tag="rbc",
                                      bufs=1)
                        for p in range(2):
                            lr = rbc[32 * p:32 * p + 1, :]
                            nc.vector.tensor_copy(o_sb[j][p * 64:(p + 1) * 64, qsl],
                                                  ps_o[p][0:64, :])
                            nc.vector.tensor_copy(lr, ps_o[p][64:65, :])
                            nc.sync.dma_start(ls_d.ap()[j, th, p, :], lr)
                            lrs = pb.tile([64, 16], F32, name="lrs",
                                          tag=f"lrs{p}", bufs=2)
                            nc.sync.dma_start(
                                lrs[:],
                                ls_d.ap()[j, th, p, :].rearrange(
                                    "(a b) -> a b", b=16))
                            rrs = pb.tile([64, 16], F32, name="rrs",
                                          tag=f"rrs{p}", bufs=2)
                            nc.vector.reciprocal(rrs[:], lrs[:])
                            nc.sync.dma_start(
                                rs_d.ap()[j, th, p, :].rearrange(
                                    "(a b) -> a b", b=16), rrs[:])
                            nc.sync.dma_start(
                                rbc[p * 64:(p + 1) * 64, :],
                                rs_d.ap()[j, th, p, :][None, :]
                                .broadcast_to((64, TQH)))
                            nc.vector.tensor_mul(
                                o_sb[j][p * 64:(p + 1) * 64, qsl],
                                o_sb[j][p * 64:(p + 1) * 64, qsl].bitcast(F32),
                                rbc[p * 64:(p + 1) * 64, :])

                pbps_ctx.__exit__(None, None, None)

                if DBG:
                    nc.sync.dma_start(dbg_q.ap(), qT[0][:].bitcast(F32))
                    for h in range(2):
                        nc.sync.dma_start(dbg_k[h].ap(), kT[h][:].bitcast(F32))
                    nc.sync.dma_start(dbg_o.ap(), o_sb[0][:].bitcast(F32))

                # ---- phase C: output projection ----
                pcps_ctx = tc.tile_pool(name="pc_ps", bufs=2, space="PSUM")
                pcps = pcps_ctx.__enter__()
                for tt in range(NTK):
                    tsl = slice(tt * 128, (tt + 1) * 128)
                    ps_y = pcps.tile([128, 1024], F32, name="py", tag="py")
                    for nk in range(2):
                        nsl = slice(nk * 512, (nk + 1) * 512)
                        for j in range(NPAIR):
                            nc.tensor.matmul(ps_y[:, nsl], o_sb[j][:, tsl],
                                             wo_t[j][:, nsl],
                                             start=(j == 0), stop=(j == NPAIR - 1))
                    y_t = pb.tile([128, 1024], F32, name="yt", tag="yt", bufs=2)
                    nc.vector.tensor_copy(y_t[:], ps_y[:])
                    nc.sync.dma_start(y_d.ap()[tsl, :], y_t[:])
                pcps_ctx.__exit__(None, None, None)

    nc.compile()
    return nc


_NC = None
_last_in_maps = None


def kernel(x, x_mean, x_std, Wq, bq, Wk, bk, Wv, bv, Wo, bo):
    global _NC
    if _NC is None:
        _NC = _build()

    x = np.asarray(x, dtype=np.float32)
    x_std = np.asarray(x_std, dtype=np.float32)
    Wq = np.asarray(Wq, dtype=np.float32)
    Wk = np.asarray(Wk, dtype=np.float32)
    Wv = np.asarray(Wv, dtype=np.float32)
    Wo = np.asarray(Wo, dtype=np.float32)
    bq = np.asarray(bq, dtype=np.float32)
    bk = np.asarray(bk, dtype=np.float32)
    bv = np.asarray(bv, dtype=np.float32)
    bo = np.asarray(bo, dtype=np.float32)

    mask = np.zeros((128, 2), dtype=np.float32)
    mask[0:64, 0] = 1.0
    mask[64:128, 1] = 1.0
    in_maps = []
    for c in range(N_CORES):
        b, g = c // 2, c % 2
        s = np.float32(SCALE / float(x_std[b, 0, 0]))
        rows = slice(g * EG, (g + 1) * EG)
        in_maps.append({
            "xt": np.ascontiguousarray(x[b].T),
            "wq": np.ascontiguousarray((Wq[rows, :] * s).T),
            "wk": np.ascontiguousarray(Wk[rows, :].T),
            "wv": np.ascontiguousarray(Wv[rows, :].T),
            "wo": np.ascontiguousarray(Wo[:, rows].T),
            "bq": np.ascontiguousarray(bq[rows] * s),
            "mask": mask,
        })

    global _last_in_maps
    _last_in_maps = in_maps
    res = run_bass_kernel_spmd(_NC, in_maps, list(range(N_CORES)))

    bias_term = (bo + bv @ Wo.T).astype(np.float32)   # [D]
    y = np.empty((B, T, D), dtype=np.float32)
    for b in range(B):
        y[b] = (res.results[2 * b]["y"] + res.results[2 * b + 1]["y"]
                + bias_term[None, :])
    return y


# revision 24
# speedup vs baseline: 1.1535x; 1.1535x over previous
"""De-stationary attention on 8 Trainium2 NeuronCores.

Problem: y = softmax((x Wq^T + bq)(x Wk^T + bk)^T * scale / (tau*x_std)) (x Wv^T + bv) Wo^T + bo
Shapes: x [4, 2048, 1024], 16 heads of 64 dims, tau=1, delta=0.

Sharding: core c handles batch b = c//2, head group g = c%2 (8 heads).
The de-stationary scale s = SCALE/x_std[b] is folded into Wq on the host.
Each core computes a partial y over its 8 heads; host sums the two
head-group partials per batch and adds bo + bv @ Wo.T (value bias passes
through softmax rows which sum to 1).

Device math per core (all matmuls float32r = TF32-like, 1 cyc/row):
  qT [512, 2048] = (s*Wq_g) x^T + s*bq  (head-pair tiles [128, 2048])
  kT [512, 2048] = Wk_g x^T + bk
  v  [2048, 8*65] : per head 64 value cols + a ones col (gives the
     softmax denominator as row 64 of the PV matmul output)
  per head pair, per tq half [1024], per tk tile [128]:
     S^T[tk, tq] = kT_h^T-slice . qT_h  (K=64, even head on PE rows 0-63,
                   odd head on rows 64-127 -> concurrent via row groups)
     P^T = exp(S^T)    (no max subtraction: |logits| <= ~25, fp32 safe)
     psum_O[65, tq] += v_aug[tk]^T . P^T   (row 64 accumulates l = sum P)
  O_norm = psum_O[0:64] * (1/l) broadcast (DVE recip + DRAM-bounce bcast)
  y_part[2048, 1024] = sum_pairs O_pair^T . wo_pair  (K=128)
"""

import os
import sys

for _p in ("/opt/trn_rl_repo", "/root/.axon_site/_ro/trn_rl_repo"):
    if os.path.isdir(_p) and _p not in sys.path:
        sys.path.insert(0, _p)

import numpy as np

import concourse.bass as bass
import concourse.mybir as mybir
import concourse.tile as tile
from concourse import bacc
from concourse.bass_utils import run_bass_kernel_spmd

F32 = mybir.dt.float32
F32R = mybir.dt.float32r
BF16 = mybir.dt.bfloat16
AF = mybir.ActivationFunctionType

B, T, D, H = 4, 2048, 1024, 16
HD = D // H          # 64
SCALE = HD ** -0.5
HG = H // 2          # 8 heads per core
EG = HG * HD         # 512 projection dims per core
N_CORES = 8

TQH = T // 2         # 1024: tq half processed per attention inner loop
NTK = T // 128       # 16 tk tiles
NPAIR = HG // 2      # 4 head pairs per core


def _build():
    nc = bacc.Bacc("TRN2", target_bir_lowering=False, debug=False)

    xt_d = nc.dram_tensor("xt", [D, T], F32, kind="ExternalInput")
    wq_d = nc.dram_tensor("wq", [D, EG], F32, kind="ExternalInput")
    wk_d = nc.dram_tensor("wk", [D, EG], F32, kind="ExternalInput")
    wv_d = nc.dram_tensor("wv", [D, EG], F32, kind="ExternalInput")
    wo_d = nc.dram_tensor("wo", [EG, D], F32, kind="ExternalInput")
    bq_d = nc.dram_tensor("bq", [EG], F32, kind="ExternalInput")
    # mask[:, 0] = rows 0:64 one / rows 64:128 zero; mask[:, 1] inverted.
    # bk needs no device handling: the q.bk logit term is constant along
    # the key axis and cancels in softmax.
    mk_d = nc.dram_tensor("mask", [128, 2], F32, kind="ExternalInput")
    y_d = nc.dram_tensor("y", [T, D], F32, kind="ExternalOutput")
    DBG = os.environ.get("KDBG", "0") == "1"
    if DBG:
        dbg_q = nc.dram_tensor("dbg_q", [128, T], F32, kind="ExternalOutput")
        dbg_k = [nc.dram_tensor(f"dbg_k{h}", [128, T], F32,
                                kind="ExternalOutput") for h in range(2)]
        dbg_o = nc.dram_tensor("dbg_o", [128, T], F32, kind="ExternalOutput")
    # scratch for the l / 1/l bounces (per head-pair iteration, per head)
    ls_d = nc.dram_tensor("l_scratch", [NPAIR, 2, 2, TQH], F32)
    rs_d = nc.dram_tensor("rinv_scratch", [NPAIR, 2, 2, TQH], F32)

    with tile.TileContext(nc) as tc:
        from contextlib import ExitStack
        with ExitStack() as octx:
            # ---- persistent pools (live through phases A-C) ----
            qk_pool = octx.enter_context(tc.tile_pool(name="qk", bufs=1))
            v_pool = octx.enter_context(tc.tile_pool(name="vp", bufs=1))

            qT = [qk_pool.tile([128, T], F32R, name=f"qT{j}", tag=f"qT{j}")
                  for j in range(NPAIR)]

            # per-head kT, zero-padded to K=128: head h occupies rows
            # (h%2)*64:(h%2+1)*64, other 64 rows are zero. Zero-padding keeps
            # the S^T matmuls at K=128 / no row-groups: K=64 row-group
            # matmuls do not register as PE activity for the HAM clock
            # governor, which then throttles the PE to 1.2 GHz.
            kT = [qk_pool.tile([128, T], F32R, name=f"kT{h}", tag=f"kT{h}")
                  for h in range(HG)]
            v_sb = [v_pool.tile([128, HG * (HD + 1)], F32R, name=f"v{t}",
                                tag=f"v{t}") for t in range(NTK)]

            # ---- warm-up: trip the PE HAM to full clock while DMAs land
            with tc.tile_pool(name="wu", bufs=1) as wup, \
                 tc.tile_pool(name="wu_ps", bufs=1, space="PSUM") as wups:
                wu_t = wup.tile([128, 512], F32R, name="wu")
                nc.vector.memset(wu_t[:].bitcast(F32), 0.0)
                wu_ps = wups.tile([128, 512], F32, name="wups")
                for i in range(80):
                    nc.tensor.matmul(wu_ps[:], wu_t[:, 0:128], wu_t[:],
                                     start=True, stop=True,
                                     skip_group_check=True)

            # ---- phase A: projections ----
            with tc.tile_pool(name="pa", bufs=1) as pa, \
                 tc.tile_pool(name="pa_x", bufs=1) as pax, \
                 tc.tile_pool(name="pa_ps", bufs=4, space="PSUM") as paps:
                wq_t = [pa.tile([128, EG], F32R, name=f"wq{k}", tag=f"wq{k}")
                        for k in range(8)]
                wk_t = [pa.tile([128, EG], F32R, name=f"wk{k}", tag=f"wk{k}")
                        for k in range(8)]
                wv_t = [pa.tile([128, EG], F32R, name=f"wv{k}", tag=f"wv{k}")
                        for k in range(8)]
                bq_t = [pa.tile([128, 1], F32, name=f"bq{e}", tag=f"bq{e}")
                        for e in range(4)]
                mk_t = pa.tile([128, 2], F32, name="mk")
                nc.sync.dma_start(mk_t[:], mk_d.ap())
                for k in range(8):
                    r = slice(k * 128, (k + 1) * 128)
                    nc.sync.dma_start(wq_t[k][:], wq_d.ap()[r, :].bitcast(F32R))
                for e in range(4):
                    nc.sync.dma_start(bq_t[e][:],
                                      bq_d.ap()[e * 128:(e + 1) * 128])
                for k in range(8):
                    r = slice(k * 128, (k + 1) * 128)
                    nc.sync.dma_start(wk_t[k][:], wk_d.ap()[r, :].bitcast(F32R))
                for k in range(8):
                    r = slice(k * 128, (k + 1) * 128)
                    nc.sync.dma_start(wv_t[k][:], wv_d.ap()[r, :].bitcast(F32R))

                for tq in range(4):          # t quarters of 512
                    tsl = slice(tq * 512, (tq + 1) * 512)
                    x_t = [pax.tile([128, 512], F32R, name=f"x{k}", tag=f"x{k}",
                                    bufs=(2 if k < 7 else 1))
                           for k in range(8)]
                    for k in range(8):
                        nc.sync.dma_start(
                            x_t[k][:],
                            xt_d.ap()[k * 128:(k + 1) * 128, tsl].bitcast(F32R))
                    # q projection: out [e-tile 128, t 512], bias bq
                    for e in range(4):
                        ps = paps.tile([128, 512], F32, name="pj", tag="pj")
                        esl = slice(e * 128, (e + 1) * 128)
                        for k in range(8):
                            nc.tensor.matmul(ps[:], wq_t[k][:, esl], x_t[k][:],
                                             start=(k == 0), stop=(k == 7))
                        nc.vector.tensor_scalar_add(qT[e][:, tsl], ps[:],
                                                    bq_t[e][:])
                    # k projection -> per-head zero-padded tiles via masks
                    for e in range(4):
                        ps = paps.tile([128, 512], F32, name="pj", tag="pj")
                        esl = slice(e * 128, (e + 1) * 128)
                        for k in range(8):
                            nc.tensor.matmul(ps[:], wk_t[k][:, esl], x_t[k][:],
                                             start=(k == 0), stop=(k == 7))
                        for p in range(2):
                            nc.vector.tensor_scalar(
                                kT[2 * e + p][:, tsl], ps[:],
                                mk_t[:, p:p + 1], None,
                                mybir.AluOpType.mult)
                    # v projection: out [tk-tile 128, e 512] -> 65-strided
                    for tt in range(4):
                        ps = paps.tile([128, 512], F32, name="pj", tag="pj")
                        ti = tq * 4 + tt
                        for k in range(8):
                            nc.tensor.matmul(
                                ps[:], x_t[k][:, tt * 128:(tt + 1) * 128],
                                wv_t[k][:], start=(k == 0), stop=(k == 7))
                        vre = v_sb[ti][:].rearrange("p (h c) -> p h c", c=HD + 1)
                        nc.vector.tensor_copy(
                            vre[:, :, 0:HD],
                            ps[:].rearrange("p (h c) -> p h c", c=HD))
                        nc.scalar.activation(vre[:, :, HD:HD + 1],
                                             ps[:].rearrange(
                                                 "p (h c) -> p h c", c=HD)[:, :, 0:1],
                                             AF.Identity, bias=1.0, scale=0.0)

            # ---- phases B+C pools ----
            with tc.tile_pool(name="pb", bufs=1) as pb:
                wo_t = [pb.tile([128, D], F32R, name=f"wo{j}", tag=f"wo{j}")
                        for j in range(NPAIR)]
                for j in range(NPAIR):
                    nc.sync.dma_start(
                        wo_t[j][:],
                        wo_d.ap()[j * 128:(j + 1) * 128, :].bitcast(F32R))
                o_sb = [pb.tile([128, T], F32R, name=f"o{j}", tag=f"o{j}")
                        for j in range(NPAIR)]

                # ---- phase B: attention ----
                pbps_ctx = tc.tile_pool(name="pb_ps", bufs=1, space="PSUM")
                pbps = pbps_ctx.__enter__()
                for j in range(NPAIR):
                    for th in range(2):
                        qsl = slice(th * TQH, (th + 1) * TQH)
                        ps_o = [pbps.tile([65, TQH], F32, name=f"po{p}",
                                          tag=f"po{p}", bufs=1)
                                for p in range(2)]
                        for tk in range(NTK):
                            ksl = slice(tk * 128, (tk + 1) * 128)
                            # S matmuls for both heads adjacent: even head on
                            # PE rows 0-63, odd on 64-127 -> run concurrently
                            ps_s = [pbps.tile([128, TQH], F32, name=f"ps{p}",
                                              tag=f"ps{p}", bufs=1)
                                    for p in range(2)]
                            for p in range(2):
                                for nk in range(2):
                                    nsl = slice(nk * 512, (nk + 1) * 512)
                                    nc.tensor.matmul(
                                        ps_s[p][:, nsl],
                                        kT[2 * j + p][:, ksl],
                                        qT[j][:, qsl][:, nsl],
                                        start=True, stop=True)
                            pt = [None, None]
                            for p in range(2):
                                pt[p] = pb.tile([128, TQH], F32R,
                                                name=f"pt{p}", tag="pt",
                                                bufs=3)
                                nc.scalar.activation(pt[p][:], ps_s[p][:], AF.Exp)
                            for p in range(2):
                                h = 2 * j + p
                                vcol = slice(h * (HD + 1), (h + 1) * (HD + 1))
                                for nk in range(2):
                                    nsl = slice(nk * 512, (nk + 1) * 512)
                                    nc.tensor.matmul(
                                        ps_o[p][:, nsl], v_sb[tk][:, vcol],
                                        pt[p][:, nsl],
                                        start=(tk == 0), stop=(tk == NTK - 1))
                        # normalize: O = psum_O[0:64] * (1/l), l = row 64.
                        # l goes SBUF->DRAM->[64,16] sbuf, recip there (fast:
                        # 16 elems/partition), back out, then bcast [64, TQH].
                        rbc = pb.tile([128, TQH], F32, name="rbc", tag="rbc",
                                      bufs=1)
                        for p in range(2):
                            lrow = pb.tile([1, TQH], F32, name="lrow",
                                           tag="lrow", bufs=1)
                            nc.vector.tensor_copy(o_sb[j][p * 64:(p + 1) * 64, qsl],
                                                  ps_o[p][0:64, :])
                            nc.vector.tensor_copy(lrow[0:1, :],
                                                  ps_o[p][64:65, :])
                            nc.sync.dma_start(ls_d.ap()[j, th, p, :],
                                              lrow[0:1, :])
                            lrs = pb.tile([64, 16], F32, name="lrs",
                                          tag=f"lrs{p}", bufs=2)
                            nc.sync.dma_start(
                                lrs[:],
                                ls_d.ap()[j, th, p, :].rearrange(
                                    "(a b) -> a b", b=16))
                            rrs = pb.tile([64, 16], F32, name="rrs",
                                          tag=f"rrs{p}", bufs=2)
                            nc.vector.reciprocal(rrs[:], lrs[:])
                            nc.sync.dma_start(
                                rs_d.ap()[j, th, p, :].rearrange(
                                    "(a b) -> a b", b=16), rrs[:])
                            nc.sync.dma_start(
                                rbc[p * 64:(p + 1) * 64, :],
                                rs_d.ap()[j, th, p, :][None, :]
                                .broadcast_to((64, TQH)))
                            nc.vector.tensor_mul(
                                o_sb[j][p * 64:(p + 1) * 64, qsl],
                                o_sb[j][p * 64:(p + 1) * 64, qsl].bitcast(F32),
                                rbc[p * 64:(p + 1) * 64, :])

                pbps_ctx.__exit__(None, None, None)

                if DBG:
                    nc.sync.dma_start(dbg_q.ap(), qT[0][:].bitcast(F32))
                    for h in range(2):
                        nc.sync.dma_start(dbg_k[h].ap(), kT[h][:].bitcast(F32))
                    nc.sync.dma_start(dbg_o.ap(), o_sb[0][:].bitcast(F32))

                # ---- phase C: output projection ----
                pcps_ctx = tc.tile_pool(name="pc_ps", bufs=2, space="PSUM")
                pcps = pcps_ctx.__enter__()
                for tt in range(NTK):
                    tsl = slice(tt * 128, (tt + 1) * 128)
                    ps_y = pcps.tile([128, 1024], F32, name="py", tag="py")
                    for nk in range(2):
                        nsl = slice(nk * 512, (nk + 1) * 512)
                        for j in range(NPAIR):
                            nc.tensor.matmul(ps_y[:, nsl], o_sb[j][:, tsl],
                                             wo_t[j][:, nsl],
                                             start=(j == 0), stop=(j == NPAIR - 1))
                    y_t = pb.tile([128, 1024], F32, name="yt", tag="yt", bufs=2)
                    nc.vector.tensor_copy(y_t[:], ps_y[:])
                    nc.sync.dma_start(y_d.ap()[tsl, :], y_t[:])
                pcps_ctx.__exit__(None, None, None)

    nc.compile()
    return nc


_NC = None
_last_in_maps = None


def kernel(x, x_mean, x_std, Wq, bq, Wk, bk, Wv, bv, Wo, bo):
    global _NC
    if _NC is None:
        _NC = _build()

    x = np.asarray(x, dtype=np.float32)
    x_std = np.asarray(x_std, dtype=np.float32)
    Wq = np.asarray(Wq, dtype=np.float32)
    Wk = np.asarray(Wk, dtype=np.float32)
    Wv = np.asarray(Wv, dtype=np.float32)
    Wo = np.asarray(Wo, dtype=np.float32)
    bq = np.asarray(bq, dtype=np.float32)
    bk = np.asarray(bk, dtype=np.float32)
    bv = np.asarray(bv, dtype=np.float32)
    bo = np.asarray(bo, dtype=np.float32)

    mask = np.zeros((128, 2), dtype=np.float32)
    mask[0:64, 0] = 1.0
    mask[64:128, 1] = 1.0
    in_maps = []
    for c in range(N_CORES):
        b, g = c // 2, c % 2
        s = np.float32(SCALE / float(x_std[b, 0, 0]))
        rows = slice(g * EG, (g + 1) * EG)
        in_maps.append({
            "xt": np.ascontiguousarray(x[b].T),
            "wq": np.ascontiguousarray((Wq[rows, :] * s).T),
            "wk": np.ascontiguousarray(Wk[rows, :].T),
            "wv": np.ascontiguousarray(Wv[rows, :].T),
            "wo": np.ascontiguousarray(Wo[:, rows].T),
            "bq": np.ascontiguousarray(bq[rows] * s),
            "mask": mask,
        })

    global _last_in_maps
    _last_in_maps = in_maps
    res = run_bass_kernel_spmd(_NC, in_maps, list(range(N_CORES)))

    bias_term = (bo + bv @ Wo.T).astype(np.float32)   # [D]
    y = np.empty((B, T, D), dtype=np.float32)
    for b in range(B):
        y[b] = (res.results[2 * b]["y"] + res.results[2 * b + 1]["y"]
                + bias_term[None, :])
    return y


# revision 25
# speedup vs baseline: 1.1556x; 1.0018x over previous
"""De-stationary attention on 8 Trainium2 NeuronCores.

Problem: y = softmax((x Wq^T + bq)(x Wk^T + bk)^T * scale / (tau*x_std)) (x Wv^T + bv) Wo^T + bo
Shapes: x [4, 2048, 1024], 16 heads of 64 dims, tau=1, delta=0.

Sharding: core c handles batch b = c//2, head group g = c%2 (8 heads).
The de-stationary scale s = SCALE/x_std[b] is folded into Wq on the host.
Each core computes a partial y over its 8 heads; host sums the two
head-group partials per batch and adds bo + bv @ Wo.T (value bias passes
through softmax rows which sum to 1).

Device math per core (all matmuls float32r = TF32-like, 1 cyc/row):
  qT [512, 2048] = (s*Wq_g) x^T + s*bq  (head-pair tiles [128, 2048])
  kT [512, 2048] = Wk_g x^T + bk
  v  [2048, 8*65] : per head 64 value cols + a ones col (gives the
     softmax denominator as row 64 of the PV matmul output)
  per head pair, per tq half [1024], per tk tile [128]:
     S^T[tk, tq] = kT_h^T-slice . qT_h  (K=64, even head on PE rows 0-63,
                   odd head on rows 64-127 -> concurrent via row groups)
     P^T = exp(S^T)    (no max subtraction: |logits| <= ~25, fp32 safe)
     psum_O[65, tq] += v_aug[tk]^T . P^T   (row 64 accumulates l = sum P)
  O_norm = psum_O[0:64] * (1/l) broadcast (DVE recip + DRAM-bounce bcast)
  y_part[2048, 1024] = sum_pairs O_pair^T . wo_pair  (K=128)
"""

import os
import sys

for _p in ("/opt/trn_rl_repo", "/root/.axon_site/_ro/trn_rl_repo"):
    if os.path.isdir(_p) and _p not in sys.path:
        sys.path.insert(0, _p)

import numpy as np

import concourse.bass as bass
import concourse.mybir as mybir
import concourse.tile as tile
from concourse import bacc
from concourse.bass_utils import run_bass_kernel_spmd

F32 = mybir.dt.float32
F32R = mybir.dt.float32r
BF16 = mybir.dt.bfloat16
AF = mybir.ActivationFunctionType

B, T, D, H = 4, 2048, 1024, 16
HD = D // H          # 64
SCALE = HD ** -0.5
HG = H // 2          # 8 heads per core
EG = HG * HD         # 512 projection dims per core
N_CORES = 8

TQH = T // 2         # 1024: tq half processed per attention inner loop
NTK = T // 128       # 16 tk tiles
NPAIR = HG // 2      # 4 head pairs per core


def _build():
    nc = bacc.Bacc("TRN2", target_bir_lowering=False, debug=False)

    xt_d = nc.dram_tensor("xt", [D, T], F32, kind="ExternalInput")
    wq_d = nc.dram_tensor("wq", [D, EG], F32, kind="ExternalInput")
    wk_d = nc.dram_tensor("wk", [D, EG], F32, kind="ExternalInput")
    wv_d = nc.dram_tensor("wv", [D, EG], F32, kind="ExternalInput")
    wo_d = nc.dram_tensor("wo", [EG, D], F32, kind="ExternalInput")
    bq_d = nc.dram_tensor("bq", [EG], F32, kind="ExternalInput")
    # mask[:, 0] = rows 0:64 one / rows 64:128 zero; mask[:, 1] inverted.
    # bk needs no device handling: the q.bk logit term is constant along
    # the key axis and cancels in softmax.
    mk_d = nc.dram_tensor("mask", [128, 2], F32, kind="ExternalInput")
    y_d = nc.dram_tensor("y", [T, D], F32, kind="ExternalOutput")
    DBG = os.environ.get("KDBG", "0") == "1"
    if DBG:
        dbg_q = nc.dram_tensor("dbg_q", [128, T], F32, kind="ExternalOutput")
        dbg_k = [nc.dram_tensor(f"dbg_k{h}", [128, T], F32,
                                kind="ExternalOutput") for h in range(2)]
        dbg_o = nc.dram_tensor("dbg_o", [128, T], F32, kind="ExternalOutput")
    # scratch for the l / 1/l bounces (per head-pair iteration, per head)
    ls_d = nc.dram_tensor("l_scratch", [NPAIR, 2, 2, TQH], F32)
    rs_d = nc.dram_tensor("rinv_scratch", [NPAIR, 2, 2, TQH], F32)

    with tile.TileContext(nc) as tc:
        from contextlib import ExitStack
        with ExitStack() as octx:
            # ---- persistent pools (live through phases A-C) ----
            qk_pool = octx.enter_context(tc.tile_pool(name="qk", bufs=1))
            v_pool = octx.enter_context(tc.tile_pool(name="vp", bufs=1))

            qT = [qk_pool.tile([128, T], F32R, name=f"qT{j}", tag=f"qT{j}")
                  for j in range(NPAIR)]

            # per-head kT, zero-padded to K=128: head h occupies rows
            # (h%2)*64:(h%2+1)*64, other 64 rows are zero. Zero-padding keeps
            # the S^T matmuls at K=128 / no row-groups: K=64 row-group
            # matmuls do not register as PE activity for the HAM clock
            # governor, which then throttles the PE to 1.2 GHz.
            kT = [qk_pool.tile([128, T], F32R, name=f"kT{h}", tag=f"kT{h}")
                  for h in range(HG)]
            v_sb = [v_pool.tile([128, HG * (HD + 1)], F32R, name=f"v{t}",
                                tag=f"v{t}") for t in range(NTK)]

            # ---- warm-up: trip the PE HAM to full clock while DMAs land
            with tc.tile_pool(name="wu", bufs=1) as wup, \
                 tc.tile_pool(name="wu_ps", bufs=1, space="PSUM") as wups:
                wu_t = wup.tile([128, 512], F32R, name="wu")
                nc.vector.memset(wu_t[:].bitcast(F32), 0.0)
                wu_ps = wups.tile([128, 512], F32, name="wups")
                for i in range(96):
                    nc.tensor.matmul(wu_ps[:], wu_t[:, 0:128], wu_t[:],
                                     start=True, stop=True,
                                     skip_group_check=True)

            # ---- phase A: projections ----
            with tc.tile_pool(name="pa", bufs=1) as pa, \
                 tc.tile_pool(name="pa_x", bufs=1) as pax, \
                 tc.tile_pool(name="pa_ps", bufs=4, space="PSUM") as paps:
                wq_t = [pa.tile([128, EG], F32R, name=f"wq{k}", tag=f"wq{k}")
                        for k in range(8)]
                wk_t = [pa.tile([128, EG], F32R, name=f"wk{k}", tag=f"wk{k}")
                        for k in range(8)]
                wv_t = [pa.tile([128, EG], F32R, name=f"wv{k}", tag=f"wv{k}")
                        for k in range(8)]
                bq_t = [pa.tile([128, 1], F32, name=f"bq{e}", tag=f"bq{e}")
                        for e in range(4)]
                for k in range(8):
                    r = slice(k * 128, (k + 1) * 128)
                    nc.sync.dma_start(wq_t[k][:], wq_d.ap()[r, :].bitcast(F32R))
                mk_t = pa.tile([128, 2], F32, name="mk")
                nc.sync.dma_start(mk_t[:], mk_d.ap())
                for e in range(4):
                    nc.sync.dma_start(bq_t[e][:],
                                      bq_d.ap()[e * 128:(e + 1) * 128])
                for k in range(8):
                    r = slice(k * 128, (k + 1) * 128)
                    nc.sync.dma_start(wk_t[k][:], wk_d.ap()[r, :].bitcast(F32R))
                for k in range(8):
                    r = slice(k * 128, (k + 1) * 128)
                    nc.sync.dma_start(wv_t[k][:], wv_d.ap()[r, :].bitcast(F32R))

                for tq in range(4):          # t quarters of 512
                    tsl = slice(tq * 512, (tq + 1) * 512)
                    x_t = [pax.tile([128, 512], F32R, name=f"x{k}", tag=f"x{k}",
                                    bufs=(2 if k < 7 else 1))
                           for k in range(8)]
                    for k in range(8):
                        nc.sync.dma_start(
                            x_t[k][:],
                            xt_d.ap()[k * 128:(k + 1) * 128, tsl].bitcast(F32R))
                    # q projection: out [e-tile 128, t 512], bias bq
                    for e in range(4):
                        ps = paps.tile([128, 512], F32, name="pj", tag="pj")
                        esl = slice(e * 128, (e + 1) * 128)
                        for k in range(8):
                            nc.tensor.matmul(ps[:], wq_t[k][:, esl], x_t[k][:],
                                             start=(k == 0), stop=(k == 7))
                        nc.vector.tensor_scalar_add(qT[e][:, tsl], ps[:],
                                                    bq_t[e][:])
                    # k projection -> per-head zero-padded tiles via masks
                    for e in range(4):
                        ps = paps.tile([128, 512], F32, name="pj", tag="pj")
                        esl = slice(e * 128, (e + 1) * 128)
                        for k in range(8):
                            nc.tensor.matmul(ps[:], wk_t[k][:, esl], x_t[k][:],
                                             start=(k == 0), stop=(k == 7))
                        for p in range(2):
                            nc.vector.tensor_scalar(
                                kT[2 * e + p][:, tsl], ps[:],
                                mk_t[:, p:p + 1], None,
                                mybir.AluOpType.mult)
                    # v projection: out [tk-tile 128, e 512] -> 65-strided
                    for tt in range(4):
                        ps = paps.tile([128, 512], F32, name="pj", tag="pj")
                        ti = tq * 4 + tt
                        for k in range(8):
                            nc.tensor.matmul(
                                ps[:], x_t[k][:, tt * 128:(tt + 1) * 128],
                                wv_t[k][:], start=(k == 0), stop=(k == 7))
                        vre = v_sb[ti][:].rearrange("p (h c) -> p h c", c=HD + 1)
                        nc.vector.tensor_copy(
                            vre[:, :, 0:HD],
                            ps[:].rearrange("p (h c) -> p h c", c=HD))
                        nc.scalar.activation(vre[:, :, HD:HD + 1],
                                             ps[:].rearrange(
                                                 "p (h c) -> p h c", c=HD)[:, :, 0:1],
                                             AF.Identity, bias=1.0, scale=0.0)

            # ---- phases B+C pools ----
            with tc.tile_pool(name="pb", bufs=1) as pb:
                wo_t = [pb.tile([128, D], F32R, name=f"wo{j}", tag=f"wo{j}")
                        for j in range(NPAIR)]
                for j in range(NPAIR):
                    nc.sync.dma_start(
                        wo_t[j][:],
                        wo_d.ap()[j * 128:(j + 1) * 128, :].bitcast(F32R))
                o_sb = [pb.tile([128, T], F32R, name=f"o{j}", tag=f"o{j}")
                        for j in range(NPAIR)]

                # ---- phase B: attention ----
                pbps_ctx = tc.tile_pool(name="pb_ps", bufs=1, space="PSUM")
                pbps = pbps_ctx.__enter__()
                for j in range(NPAIR):
                    for th in range(2):
                        qsl = slice(th * TQH, (th + 1) * TQH)
                        ps_o = [pbps.tile([65, TQH], F32, name=f"po{p}",
                                          tag=f"po{p}", bufs=1)
                                for p in range(2)]
                        for tk in range(NTK):
                            ksl = slice(tk * 128, (tk + 1) * 128)
                            # S matmuls for both heads adjacent: even head on
                            # PE rows 0-63, odd on 64-127 -> run concurrently
                            ps_s = [pbps.tile([128, TQH], F32, name=f"ps{p}",
                                              tag=f"ps{p}", bufs=1)
                                    for p in range(2)]
                            for p in range(2):
                                for nk in range(2):
                                    nsl = slice(nk * 512, (nk + 1) * 512)
                                    nc.tensor.matmul(
                                        ps_s[p][:, nsl],
                                        kT[2 * j + p][:, ksl],
                                        qT[j][:, qsl][:, nsl],
                                        start=True, stop=True)
                            pt = [None, None]
                            for p in range(2):
                                pt[p] = pb.tile([128, TQH], F32R,
                                                name=f"pt{p}", tag="pt",
                                                bufs=3)
                                nc.scalar.activation(pt[p][:], ps_s[p][:], AF.Exp)
                            for p in range(2):
                                h = 2 * j + p
                                vcol = slice(h * (HD + 1), (h + 1) * (HD + 1))
                                for nk in range(2):
                                    nsl = slice(nk * 512, (nk + 1) * 512)
                                    nc.tensor.matmul(
                                        ps_o[p][:, nsl], v_sb[tk][:, vcol],
                                        pt[p][:, nsl],
                                        start=(tk == 0), stop=(tk == NTK - 1))
                        # normalize: O = psum_O[0:64] * (1/l), l = row 64.
                        # l goes SBUF->DRAM->[64,16] sbuf, recip there (fast:
                        # 16 elems/partition), back out, then bcast [64, TQH].
                        rbc = pb.tile([128, TQH], F32, name="rbc", tag="rbc",
                                      bufs=1)
                        for p in range(2):
                            lrow = pb.tile([1, TQH], F32, name="lrow",
                                           tag="lrow", bufs=1)
                            nc.vector.tensor_copy(o_sb[j][p * 64:(p + 1) * 64, qsl],
                                                  ps_o[p][0:64, :])
                            nc.vector.tensor_copy(lrow[0:1, :],
                                                  ps_o[p][64:65, :])
                            nc.sync.dma_start(ls_d.ap()[j, th, p, :],
                                              lrow[0:1, :])
                            lrs = pb.tile([64, 16], F32, name="lrs",
                                          tag=f"lrs{p}", bufs=2)
                            nc.sync.dma_start(
                                lrs[:],
                                ls_d.ap()[j, th, p, :].rearrange(
                                    "(a b) -> a b", b=16))
                            rrs = pb.tile([64, 16], F32, name="rrs",
                                          tag=f"rrs{p}", bufs=2)
                            nc.vector.reciprocal(rrs[:], lrs[:])
                            nc.sync.dma_start(
                                rs_d.ap()[j, th, p, :].rearrange(
                                    "(a b) -> a b", b=16), rrs[:])
                            nc.sync.dma_start(
                                rbc[p * 64:(p + 1) * 64, :],
                                rs_d.ap()[j, th, p, :][None, :]
                                .broadcast_to((64, TQH)))
                            nc.vector.tensor_mul(
                                o_sb[j][p * 64:(p + 1) * 64, qsl],
                                o_sb[j][p * 64:(p + 1) * 64, qsl].bitcast(F32),
                                rbc[p * 64:(p + 1) * 64, :])

                pbps_ctx.__exit__(None, None, None)

                if DBG:
                    nc.sync.dma_start(dbg_q.ap(), qT[0][:].bitcast(F32))
                    for h in range(2):
                        nc.sync.dma_start(dbg_k[h].ap(), kT[h][:].bitcast(F32))
                    nc.sync.dma_start(dbg_o.ap(), o_sb[0][:].bitcast(F32))

                # ---- phase C: output projection ----
                pcps_ctx = tc.tile_pool(name="pc_ps", bufs=2, space="PSUM")
                pcps = pcps_ctx.__enter__()
                for tt in range(NTK):
                    tsl = slice(tt * 128, (tt + 1) * 128)
                    ps_y = pcps.tile([128, 1024], F32, name="py", tag="py")
                    for nk in range(2):
                        nsl = slice(nk * 512, (nk + 1) * 512)
                        for j in range(NPAIR):
                            nc.tensor.matmul(ps_y[:, nsl], o_sb[j][:, tsl],
                                             wo_t[j][:, nsl],
                                             start=(j == 0), stop=(j == NPAIR - 1))
                    y_t = pb.tile([128, 1024], F32, name="yt", tag="yt", bufs=2)
                    nc.scalar.copy(y_t[:], ps_y[:])
                    nc.sync.dma_start(y_d.ap()[tsl, :], y_t[:])
                pcps_ctx.__exit__(None, None, None)

    nc.compile()
    return nc


_NC = None
_last_in_maps = None


def kernel(x, x_mean, x_std, Wq, bq, Wk, bk, Wv, bv, Wo, bo):
    global _NC
    if _NC is None:
        _NC = _build()

    x = np.asarray(x, dtype=np.float32)
    x_std = np.asarray(x_std, dtype=np.float32)
    Wq = np.asarray(Wq, dtype=np.float32)
    Wk = np.asarray(Wk, dtype=np.float32)
    Wv = np.asarray(Wv, dtype=np.float32)
    Wo = np.asarray(Wo, dtype=np.float32)
    bq = np.asarray(bq, dtype=np.float32)
    bk = np.asarray(bk, dtype=np.float32)
    bv = np.asarray(bv, dtype=np.float32)
    bo = np.asarray(bo, dtype=np.float32)

    mask = np.zeros((128, 2), dtype=np.float32)
    mask[0:64, 0] = 1.0
    mask[64:128, 1] = 1.0
    in_maps = []
    for c in range(N_CORES):
        b, g = c // 2, c % 2
        s = np.float32(SCALE / float(x_std[b, 0, 0]))
        rows = slice(g * EG, (g + 1) * EG)
        in_maps.append({
            "xt": np.ascontiguousarray(x[b].T),
            "wq": np.ascontiguousarray((Wq[rows, :] * s).T),
            "wk": np.ascontiguousarray(Wk[rows, :].T),
            "wv": np.ascontiguousarray(Wv[rows, :].T),
            "wo": np.ascontiguousarray(Wo[:, rows].T),
            "bq": np.ascontiguousarray(bq[rows] * s),
            "mask": mask,
        })

    global _last_in_maps
    _last_in_maps = in_maps
    res = run_bass_kernel_spmd(_NC, in_maps, list(range(N_CORES)))

    bias_term = (bo + bv @ Wo.T).astype(np.float32)   # [D]
    y = np.empty((B, T, D), dtype=np.float32)
    for b in range(B):
        y[b] = (res.results[2 * b]["y"] + res.results[2 * b + 1]["y"]
                + bias_term[None, :])
    return y
